# revision 1
# baseline (speedup 1.0000x reference)
"""MiniGPT2 forward pass on 8 Trainium2 NeuronCores (Bass/Tile).

Sharding: tokens are stride-8 interleaved across all 8 cores -- core c owns
tokens {8i+c} of BOTH batch elements (128 tokens each, 256 total).  Causal
attention work is then identical on every core (uniform SPMD program, with
per-core causality expressed in 0/1 mask *data*), and the only collective is
a full 8-rank AllGather of locally-computed K/V per layer.  The tied LM head
is vocab-sharded 8 ways after one final AllGather of activations.

All activations live in transposed [feature, token] layout so no on-device
transposes are ever needed; every DMA is an identity [128, F] copy (the host
pre-arranges all tensors into their exact SBUF image).  Matmuls run in bf16
with fp32 PSUM accumulation and an fp32 residual stream.
"""

import sys

if "/opt/trn_rl_repo" not in sys.path:
    sys.path.insert(0, "/opt/trn_rl_repo")

import numpy as np
import ml_dtypes

BF16 = ml_dtypes.bfloat16

# Model config (hardcoded per problem spec)
V = 50257
D = 1024
H = 16
HD = 64
L = 8
FF = 4096
B = 2
T = 1024
SCALE = 1.0 / 8.0
EPS = 1e-5

N_CORES = 8
TL = 256         # local tokens per core (128 per batch element)
DT = D // 128    # 8 D-tiles
HP = H // 2      # 8 head-pairs
FT1 = FF // 128  # 32 fc1 output tiles
NV = 13          # vocab chunks of 512 per core
VPAD = NV * 512  # 6656 padded vocab slice per core
RG = [[0, 1, 2, 3, 4, 5, 6, 7]]

# packed per-layer bias/param tile columns (f32 [128, 104])
BC_QKVB = 0      # 24 cols
BC_PROJB = 24    # 8
BC_FC1B = 32     # 32
BC_FC2B = 64     # 8
BC_LN1W = 72     # 8
BC_LN1B = 80
BC_LN2W = 88
BC_LN2B = 96
BCOLS = 104


def _build_program(n_layers=L, nv=NV, debug=False):
    import concourse.mybir as mybir
    import concourse.tile as tile
    from concourse import bacc
    from contextlib import ExitStack

    f32 = mybir.dt.float32
    bf16 = mybir.dt.bfloat16
    AF = mybir.ActivationFunctionType
    OP = mybir.AluOpType

    nc = bacc.Bacc("TRN2", target_bir_lowering=False, debug=False,
                   num_devices=N_CORES)

    # ---- external I/O (all pre-arranged host-side as SBUF images) ----
    x0T = nc.dram_tensor("x0T", [128, DT * TL], f32, kind="ExternalInput")
    qkwT = nc.dram_tensor("qkwT", [n_layers, 16, 128, DT * 128], bf16, kind="ExternalInput")
    vwT = nc.dram_tensor("vwT", [n_layers, DT, 128, 1024], bf16, kind="ExternalInput")
    projwT = nc.dram_tensor("projwT", [n_layers, DT, 128, DT * 128], bf16, kind="ExternalInput")
    fc1wT = nc.dram_tensor("fc1wT", [n_layers, FT1, 128, DT * 128], bf16, kind="ExternalInput")
    fc2wT = nc.dram_tensor("fc2wT", [n_layers, DT, 128, FT1 * 128], bf16, kind="ExternalInput")
    bias_in = nc.dram_tensor("biases", [n_layers, 128, BCOLS], f32, kind="ExternalInput")
    lnf_in = nc.dram_tensor("lnf", [128, 16], f32, kind="ExternalInput")
    masks_in = nc.dram_tensor("masks01", [128, 8 * 128], bf16, kind="ExternalInput")
    embT = nc.dram_tensor("embT", [nv, 128, DT * 512], bf16, kind="ExternalInput")
    out = nc.dram_tensor("out", [2 * 8, nv, 128, 512], f32, kind="ExternalOutput")
    dbg = {}
    if debug:
        for nm, shape, dt_ in [
            ("dbg_h1", [128, DT * TL], bf16), ("dbg_q", [128, HP * TL], bf16),
            ("dbg_k", [128, HP * TL], bf16), ("dbg_vloc", [128, 2 * H * 65], bf16),
            ("dbg_kall0", [128, 8 * 1024], bf16), ("dbg_vtall0", [128, 8 * H * 65], bf16),
            ("dbg_yT", [128, HP * TL], bf16), ("dbg_x1", [128, DT * TL], f32),
            ("dbg_g", [128, FT1 * TL], bf16), ("dbg_x2", [128, DT * TL], f32),
        ]:
            dbg[nm] = nc.dram_tensor(nm, shape, dt_, kind="ExternalOutput")

    with tile.TileContext(nc) as tc:
        with ExitStack() as ctx:
            pool = lambda *a, **k: ctx.enter_context(tc.tile_pool(*a, **k))
            p_const = pool(name="const", bufs=1)
            p_x = pool(name="xres", bufs=1)
            p_h = pool(name="h", bufs=1)
            p_qk = pool(name="qk", bufs=1)
            p_vloc = pool(name="vloc", bufs=1)
            p_kvall = pool(name="kvall", bufs=1)
            p_y = pool(name="y", bufs=1)
            p_g = pool(name="g", bufs=1)
            p_wqk = pool(name="wqk", bufs=2)
            p_wv = pool(name="wv", bufs=2)
            p_wproj = pool(name="wproj", bufs=2)
            p_wfc1 = pool(name="wfc1", bufs=2)
            p_wfc2 = pool(name="wfc2", bufs=2)
            p_wemb = pool(name="wemb", bufs=3)
            p_bias = pool(name="bias", bufs=1)
            p_stat = pool(name="stat", bufs=1)
            p_ab = pool(name="ab", bufs=1)
            p_anr = pool(name="anr", bufs=1)
            p_scr = pool(name="scratch", bufs=3)
            ps_mm = pool(name="ps_mm", bufs=4, space="PSUM")
            ps_s = pool(name="ps_s", bufs=2, space="PSUM")
            ps_o = pool(name="ps_o", bufs=2, space="PSUM")
            p_dram = pool(name="dram", bufs=2, space="DRAM")

            # ---- preamble: constants ----
            cst = p_const.tile([128, 8 * 128 + 1], bf16)  # masks | ones col
            masks = cst[:, 0:8 * 128]
            ones_ln = cst[:, 8 * 128:8 * 128 + 1]
            nc.sync.dma_start(masks, masks_in.ap())
            nc.vector.memset(ones_ln, 1.0)
            lnf = p_const.tile([128, 16], f32)
            nc.sync.dma_start(lnf[:], lnf_in.ap())

            x = p_x.tile([128, DT * TL], f32, tag="x")
            nc.sync.dma_start(x[:], x0T.ap())

            def layernorm(x_t, w_ap, b_ap):
                """x_t: [128, DT*TL] f32 transposed resid -> new bf16 tile."""
                stats1 = ps_mm.tile([1, TL], f32, tag="mm")
                stats2 = ps_mm.tile([1, TL], f32, tag="mm")
                for d in range(DT):
                    sc = p_scr.tile([128, 2 * TL], bf16, tag="sc16")
                    xb, xq = sc[:, 0:TL], sc[:, TL:2 * TL]
                    nc.scalar.copy(xb, x_t[:, d * TL:(d + 1) * TL])
                    nc.scalar.square(xq, x_t[:, d * TL:(d + 1) * TL])
                    nc.tensor.matmul(stats1[:], ones_ln, xb,
                                     start=(d == 0), stop=(d == DT - 1))
                    nc.tensor.matmul(stats2[:], ones_ln, xq,
                                     start=(d == 0), stop=(d == DT - 1))
                # st cols: mu 0:TL | ex2 TL:2TL | var/sd 2TL:3TL | rstd 3TL:4TL
                st = p_stat.tile([1, 4 * TL], f32, tag="st")
                mu, ex2 = st[:, 0:TL], st[:, TL:2 * TL]
                vsd, rstd = st[:, 2 * TL:3 * TL], st[:, 3 * TL:4 * TL]
                nc.vector.tensor_scalar_mul(mu, stats1[:], 1.0 / D)
                nc.vector.tensor_scalar_mul(ex2, stats2[:], 1.0 / D)
                nc.vector.scalar_tensor_tensor(
                    vsd, mu, -1.0, mu, op0=OP.mult, op1=OP.mult)
                nc.vector.scalar_tensor_tensor(
                    vsd, vsd, EPS, ex2, op0=OP.add, op1=OP.add)
                nc.scalar.activation(vsd, vsd, AF.Sqrt, bias=0.0, scale=1.0)
                nc.vector.reciprocal(rstd, vsd)
                # cc = -mu * rstd overwrites mu
                nc.vector.scalar_tensor_tensor(
                    mu, mu, -1.0, rstd, op0=OP.mult, op1=OP.mult)
                ab = p_ab.tile([128, 2 * TL], f32, tag="ab")
                nc.gpsimd.partition_broadcast(ab[:, 0:TL], rstd)
                nc.gpsimd.partition_broadcast(ab[:, TL:2 * TL], mu)
                h_t = p_h.tile([128, DT * TL], bf16, tag="h")
                for d in range(DT):
                    sl = slice(d * TL, (d + 1) * TL)
                    tmp = p_scr.tile([128, TL], f32, tag="tmp32")
                    nc.vector.tensor_mul(tmp[:], x_t[:, sl], ab[:, 0:TL])
                    nc.vector.tensor_add(tmp[:], tmp[:], ab[:, TL:2 * TL])
                    nc.scalar.activation(
                        h_t[:, sl], tmp[:], AF.Identity,
                        bias=b_ap[:, d:d + 1], scale=w_ap[:, d:d + 1])
                return h_t

            for l in range(n_layers):
                bias = p_bias.tile([128, BCOLS], f32, tag="bias")
                nc.sync.dma_start(bias[:], bias_in.ap()[l])

                h1 = layernorm(x, bias[:, BC_LN1W:BC_LN1W + 8],
                               bias[:, BC_LN1B:BC_LN1B + 8])

                if debug and l == 0:
                    nc.sync.dma_start(dbg["dbg_h1"].ap(), h1[:])
                # ---- Q, K (in [head_dim, token] layout) ----
                q_sb = p_qk.tile([128, HP * TL], bf16, tag="q")
                k_sb = p_qk.tile([128, HP * TL], bf16, tag="k")
                for f in range(16):
                    wt = p_wqk.tile([128, DT * 128], bf16, tag="wqk")
                    nc.sync.dma_start(wt[:], qkwT.ap()[l, f])
                    ps = ps_mm.tile([128, TL], f32, tag="mm")
                    for d in range(DT):
                        nc.tensor.matmul(
                            ps[:], wt[:, d * 128:(d + 1) * 128],
                            h1[:, d * TL:(d + 1) * TL],
                            start=(d == 0), stop=(d == DT - 1))
                    dst = q_sb if f < HP else k_sb
                    fo = (f % HP) * TL
                    nc.scalar.activation(
                        dst[:, fo:fo + TL], ps[:], AF.Identity,
                        bias=bias[:, BC_QKVB + f:BC_QKVB + f + 1], scale=1.0)

                # ---- V in [token, head_dim] layout with interleaved ones ----
                # v_loc cols: per b (1040): per head h (65): 64 v | 1 one
                v_loc = p_vloc.tile([128, 2 * H * 65], bf16, tag="vloc")
                nc.vector.memset(
                    v_loc[:].rearrange("p (k c) -> p k c", c=65)[:, :, 64:65], 1.0)
                ps_v = [ps_mm.tile([128, 512], f32, tag="mm", name=f"ps_v{i}")
                        for i in range(4)]
                for d in range(DT):
                    wv = p_wv.tile([128, 1024], bf16, tag="wv")
                    nc.sync.dma_start(wv[:], vwT.ap()[l, d])
                    for b in range(2):
                        for hf in range(2):
                            nc.tensor.matmul(
                                ps_v[b * 2 + hf],
                                h1[:, d * TL + b * 128: d * TL + b * 128 + 128],
                                wv[:, hf * 512:(hf + 1) * 512],
                                start=(d == 0), stop=(d == DT - 1))
                for b in range(2):
                    for hf in range(2):
                        dst = v_loc[:, b * H * 65 + hf * 8 * 65:
                                    b * H * 65 + (hf * 8 + 8) * 65]
                        nc.scalar.copy(
                            dst.rearrange("p (h c) -> p h c", c=65)[:, :, 0:64],
                            ps_v[b * 2 + hf][:].rearrange(
                                "p (h c) -> p h c", c=64))

                if debug and l == 0:
                    nc.sync.dma_start(dbg["dbg_q"].ap(), q_sb[:])
                    nc.sync.dma_start(dbg["dbg_k"].ap(), k_sb[:])
                    nc.sync.dma_start(dbg["dbg_vloc"].ap(), v_loc[:])
                # ---- AllGather K and V (identity [128,F] -> [8*128,F]) ----
                k_in = p_dram.tile([128, HP * TL], bf16, tag="k_in")
                nc.sync.dma_start(k_in[:], k_sb[:])
                v_in = p_dram.tile([128, 2 * H * 65], bf16, tag="v_in")
                nc.sync.dma_start(v_in[:], v_loc[:])
                k_out = p_dram.tile([8 * 128, HP * TL], bf16, tag="k_out")
                v_out = p_dram.tile([8 * 128, 2 * H * 65], bf16, tag="v_out")
                nc.gpsimd.collective_compute(
                    "AllGather", OP.bypass, ins=[k_in.opt()], outs=[k_out.opt()],
                    replica_groups=RG)
                nc.gpsimd.collective_compute(
                    "AllGather", OP.bypass, ins=[v_in.opt()], outs=[v_out.opt()],
                    replica_groups=RG)

                # ---- attention, one batch element at a time ----
                yT = p_y.tile([128, HP * TL], bf16, tag="yT")
                for b in range(2):
                    # k_all cols: per rank rr (1024): per hp (128)
                    k_all = p_kvall.tile([128, 8 * 1024], bf16, tag="k_all",
                                         name=f"k_all_{l}_{b}")
                    for rr in range(8):
                        nc.sync.dma_start(
                            k_all[:, rr * 1024:(rr + 1) * 1024]
                            .rearrange("p (f t) -> p f t", t=128),
                            k_out[rr * 128:(rr + 1) * 128]
                            .rearrange("p (f t) -> p f t", t=TL)[:, :, b * 128:(b + 1) * 128])
                    # vT_all cols: per rank rr (1040): per head h (65)
                    vT_all = p_kvall.tile([128, 8 * H * 65], bf16, tag="vT_all",
                                          name=f"vT_all_{l}_{b}")
                    for rr in range(8):
                        nc.sync.dma_start(
                            vT_all[:, rr * H * 65:(rr + 1) * H * 65],
                            v_out[rr * 128:(rr + 1) * 128,
                                        b * H * 65:(b + 1) * H * 65])
                    if debug and l == 0 and b == 0:
                        nc.sync.dma_start(dbg["dbg_kall0"].ap(), k_all[:])
                        nc.sync.dma_start(dbg["dbg_vtall0"].ap(), vT_all[:])
                    for h in range(H):
                        po = 64 * (h % 2)
                        hp = h // 2
                        o_ps = ps_o.tile([65, 128], f32, tag="o")
                        for rr in range(8):
                            s_ps = ps_s.tile([128, 128], f32, tag="s")
                            nc.tensor.matmul(
                                s_ps[:],
                                k_all[po:po + 64, rr * 1024 + hp * 128:
                                      rr * 1024 + hp * 128 + 128],
                                q_sb[po:po + 64, hp * TL + b * 128:
                                     hp * TL + b * 128 + 128],
                                start=True, stop=True)
                            p_sb = p_scr.tile([128, 128], bf16, tag="sc16")
                            nc.scalar.activation(p_sb[:], s_ps[:], AF.Exp,
                                                 bias=0.0, scale=SCALE)
                            nc.vector.tensor_mul(
                                p_sb[:], p_sb[:],
                                masks[:, rr * 128:(rr + 1) * 128])
                            nc.tensor.matmul(
                                o_ps[:],
                                vT_all[:, (rr * H + h) * 65:(rr * H + h) * 65 + 65],
                                p_sb[:],
                                start=(rr == 0), stop=(rr == 7))
                        anr_r = p_anr.tile([1, 128], f32, tag="anr_r")
                        nc.vector.reciprocal(anr_r[:], o_ps[64:65, :])
                        anr_b = p_anr.tile([64, 128], f32, tag="anr_b")
                        nc.gpsimd.partition_broadcast(anr_b[:], anr_r[:])
                        ysl = yT[po:po + 64, hp * TL + b * 128: hp * TL + b * 128 + 128]
                        nc.vector.tensor_mul(ysl, o_ps[0:64, :], anr_b[:])
                        nc.scalar.activation(
                            ysl, ysl, AF.Identity,
                            bias=bias[po:po + 64, BC_QKVB + 16 + hp:BC_QKVB + 17 + hp],
                            scale=1.0)

                if debug and l == 0:
                    nc.sync.dma_start(dbg["dbg_yT"].ap(), yT[:])
                # ---- proj + residual (in place on x) ----
                for f in range(DT):
                    wt = p_wproj.tile([128, DT * 128], bf16, tag="wproj")
                    nc.sync.dma_start(wt[:], projwT.ap()[l, f])
                    ps = ps_mm.tile([128, TL], f32, tag="mm")
                    for k in range(DT):
                        nc.tensor.matmul(
                            ps[:], wt[:, k * 128:(k + 1) * 128],
                            yT[:, k * TL:(k + 1) * TL],
                            start=(k == 0), stop=(k == DT - 1))
                    sl = slice(f * TL, (f + 1) * TL)
                    nc.vector.scalar_tensor_tensor(
                        x[:, sl], ps[:], bias[:, BC_PROJB + f:BC_PROJB + f + 1],
                        x[:, sl], op0=OP.add, op1=OP.add)

                if debug and l == 0:
                    nc.sync.dma_start(dbg["dbg_x1"].ap(), x[:])
                # ---- MLP ----
                h2 = layernorm(x, bias[:, BC_LN2W:BC_LN2W + 8],
                               bias[:, BC_LN2B:BC_LN2B + 8])
                gT = p_g.tile([128, FT1 * TL], bf16, tag="gT")
                for f in range(FT1):
                    wt = p_wfc1.tile([128, DT * 128], bf16, tag="wfc1")
                    nc.sync.dma_start(wt[:], fc1wT.ap()[l, f])
                    ps = ps_mm.tile([128, TL], f32, tag="mm")
                    for d in range(DT):
                        nc.tensor.matmul(
                            ps[:], wt[:, d * 128:(d + 1) * 128],
                            h2[:, d * TL:(d + 1) * TL],
                            start=(d == 0), stop=(d == DT - 1))
                    nc.scalar.activation(
                        gT[:, f * TL:(f + 1) * TL], ps[:], AF.Gelu,
                        bias=bias[:, BC_FC1B + f:BC_FC1B + f + 1], scale=1.0)
                if debug and l == 0:
                    nc.sync.dma_start(dbg["dbg_g"].ap(), gT[:])
                for f in range(DT):
                    ps = ps_mm.tile([128, TL], f32, tag="mm")
                    for kg in range(2):
                        wt = p_wfc2.tile([128, 16 * 128], bf16, tag="wfc2")
                        nc.sync.dma_start(
                            wt[:], fc2wT.ap()[l, f][:, kg * 2048:(kg + 1) * 2048])
                        for k in range(16):
                            kk = kg * 16 + k
                            nc.tensor.matmul(
                                ps[:], wt[:, k * 128:(k + 1) * 128],
                                gT[:, kk * TL:(kk + 1) * TL],
                                start=(kk == 0), stop=(kk == FT1 - 1))
                    sl = slice(f * TL, (f + 1) * TL)
                    nc.vector.scalar_tensor_tensor(
                        x[:, sl], ps[:], bias[:, BC_FC2B + f:BC_FC2B + f + 1],
                        x[:, sl], op0=OP.add, op1=OP.add)

            if debug:
                nc.sync.dma_start(dbg["dbg_x2"].ap(), x[:])
            # ---- final LN + AllGather + vocab-sharded tied head ----
            hf = layernorm(x, lnf[:, 0:8], lnf[:, 8:16])
            hf_in = p_dram.tile([128, DT * TL], bf16, tag="hf_in")
            nc.sync.dma_start(hf_in[:], hf[:])
            hf_out = p_dram.tile([8 * 128, DT * TL], bf16, tag="hf_out")
            nc.gpsimd.collective_compute(
                "AllGather", OP.bypass, ins=[hf_in.opt()], outs=[hf_out.opt()],
                replica_groups=RG)

            for b in range(2):
                # hf_all cols: per rank rr (1024): per d (128)
                hf_all = p_kvall.tile([128, 8 * 1024], bf16, tag="k_all",
                                      name=f"hf_all_{b}")
                for rr in range(8):
                    nc.sync.dma_start(
                        hf_all[:, rr * 1024:(rr + 1) * 1024]
                        .rearrange("p (f t) -> p f t", t=128),
                        hf_out[rr * 128:(rr + 1) * 128]
                        .rearrange("p (f t) -> p f t", t=TL)[:, :, b * 128:(b + 1) * 128])
                for v in range(nv):
                    ets = []
                    for g in range(2):
                        et = p_wemb.tile([128, 4 * 512], bf16, tag="emb",
                                         name=f"et_{b}_{v}_{g}")
                        nc.sync.dma_start(
                            et[:], embT.ap()[v][:, g * 2048:(g + 1) * 2048])
                        ets.append(et)
                    for t in range(8):
                        ps = ps_mm.tile([128, 512], f32, tag="mm")
                        for d in range(DT):
                            nc.tensor.matmul(
                                ps[:],
                                hf_all[:, t * 1024 + d * 128: t * 1024 + d * 128 + 128],
                                ets[d // 4][:, (d % 4) * 512:(d % 4 + 1) * 512],
                                start=(d == 0), stop=(d == DT - 1))
                        osb = p_scr.tile([128, 512], f32, tag="tmp32")
                        nc.scalar.copy(osb[:], ps[:])
                        nc.sync.dma_start(out.ap()[b * 8 + t, v], osb[:])

    nc.compile()
    return nc


def _sbuf_image(wT, ft):
    """[K, F] (already transposed weight) -> [F//ft, 128, (K//128)*ft]."""
    K, F = wT.shape
    return np.ascontiguousarray(
        wT.reshape(K // 128, 128, F // ft, ft).transpose(2, 1, 0, 3)
        .reshape(F // ft, 128, (K // 128) * ft))


def prep_inputs(inputs, n_layers=L, nv=NV):
    """Build the 8 per-core input maps from full model inputs."""
    idx = np.asarray(inputs["idx"]).astype(np.int64)
    tok_emb = np.asarray(inputs["tok_emb"], dtype=np.float32)
    pos_emb = np.asarray(inputs["pos_emb"], dtype=np.float32)
    x0 = tok_emb[idx] + pos_emb[0, :T][None, :, :]    # [B, T, D] f32

    vpad = nv * 512
    emb_pad = np.zeros((N_CORES * vpad, D), dtype=np.float32)
    emb_pad[:min(N_CORES * vpad, V)] = tok_emb[:min(N_CORES * vpad, V)]

    shared = {}
    qkw = np.empty((n_layers, 16, 128, DT * 128), dtype=BF16)
    vw = np.empty((n_layers, DT, 128, 1024), dtype=BF16)
    for l in range(n_layers):
        wT = np.asarray(inputs["qkv_w"][l], dtype=np.float32).T  # [D, 3D]
        qkw[l] = _sbuf_image(wT[:, :2 * D].astype(BF16), 128)
        vw[l] = wT[:, 2 * D:].astype(BF16).reshape(DT, 128, 1024)
    shared["qkwT"] = qkw
    shared["vwT"] = vw
    shared["projwT"] = np.stack([
        _sbuf_image(np.asarray(inputs["proj_w"][l], dtype=np.float32).T.astype(BF16), 128)
        for l in range(n_layers)])
    shared["fc1wT"] = np.stack([
        _sbuf_image(np.asarray(inputs["fc1_w"][l], dtype=np.float32).T.astype(BF16), 128)
        for l in range(n_layers)])
    shared["fc2wT"] = np.stack([
        _sbuf_image(np.asarray(inputs["fc2_w"][l], dtype=np.float32).T.astype(BF16), 128)
        for l in range(n_layers)])

    def cols(name, n):
        a = np.asarray(inputs[name], dtype=np.float32)[:n_layers]
        return a.reshape(n_layers, n, 128).transpose(0, 2, 1)

    biases = np.zeros((n_layers, 128, BCOLS), dtype=np.float32)
    biases[:, :, BC_QKVB:BC_QKVB + 24] = cols("qkv_b", 24)
    biases[:, :, BC_PROJB:BC_PROJB + 8] = cols("proj_b", 8)
    biases[:, :, BC_FC1B:BC_FC1B + 32] = cols("fc1_b", 32)
    biases[:, :, BC_FC2B:BC_FC2B + 8] = cols("fc2_b", 8)
    biases[:, :, BC_LN1W:BC_LN1W + 8] = cols("ln1_w", 8)
    biases[:, :, BC_LN1B:BC_LN1B + 8] = cols("ln1_b", 8)
    biases[:, :, BC_LN2W:BC_LN2W + 8] = cols("ln2_w", 8)
    biases[:, :, BC_LN2B:BC_LN2B + 8] = cols("ln2_b", 8)
    shared["biases"] = np.ascontiguousarray(biases)
    lnf = np.zeros((128, 16), dtype=np.float32)
    lnf[:, 0:8] = np.asarray(inputs["lnf_w"], dtype=np.float32).reshape(8, 128).T
    lnf[:, 8:16] = np.asarray(inputs["lnf_b"], dtype=np.float32).reshape(8, 128).T
    shared["lnf"] = lnf

    tri_incl = np.tril(np.ones((128, 128), dtype=np.float32)).T  # [kt,q] kt<=q
    tri_excl = np.tril(np.ones((128, 128), dtype=np.float32), -1).T  # kt<q

    ii = np.arange(128)
    in_maps = []
    for c in range(N_CORES):
        m = dict(shared)
        xl = np.concatenate([x0[0, 8 * ii + c], x0[1, 8 * ii + c]], 0)  # [256, D]
        m["x0T"] = np.ascontiguousarray(
            xl.T.reshape(DT, 128, TL).transpose(1, 0, 2).reshape(128, DT * TL))
        mk = np.concatenate(
            [tri_incl if rr <= c else tri_excl for rr in range(8)], 1)
        m["masks01"] = np.ascontiguousarray(mk.astype(BF16))
        esl = emb_pad[c * vpad:(c + 1) * vpad]        # [vpad, D]
        m["embT"] = _sbuf_image(esl.T.astype(BF16), 512)
        in_maps.append(m)
    return in_maps


def assemble_output(results, nv=NV):
    """results: list of 8 dicts with 'out' [16, nv, 128, 512] f32."""
    vpad = nv * 512
    logits = np.empty((B, T, V), dtype=np.float32)
    rows128 = np.arange(128)
    for c in range(N_CORES):
        o = results[c]["out"]                        # [16, nv, 128, 512]
        o = o.transpose(0, 2, 1, 3).reshape(16, 128, vpad)
        lo = c * vpad
        hi = min((c + 1) * vpad, V)
        if hi <= lo:
            continue
        for b in range(B):
            for t in range(8):
                g = 8 * rows128 + t
                logits[b, g, lo:hi] = o[b * 8 + t, :, :hi - lo]
    return logits


_prog_cache = {}


def _get_program(n_layers=L, nv=NV, debug=False):
    key = (n_layers, nv, debug)
    if key not in _prog_cache:
        _prog_cache[key] = _build_program(n_layers, nv, debug)
    return _prog_cache[key]


def run(inputs, n_layers=L, nv=NV, trace=False, debug=False):
    from concourse import bass_utils
    nc = _get_program(n_layers, nv, debug)
    in_maps = prep_inputs(inputs, n_layers, nv)
    res = bass_utils.run_bass_kernel_spmd(
        nc, in_maps, core_ids=list(range(N_CORES)), trace=trace)
    return assemble_output(res.results, nv), res


def kernel(**inputs):
    logits, _ = run(inputs)
    return logits



# revision 73
# speedup vs baseline: 1.7308x; 1.7308x over previous
"""MiniGPT2 forward pass on 8 Trainium2 NeuronCores (Bass/Tile).

Sharding: tokens are stride-8 interleaved across all 8 cores -- core c owns
tokens {8i+c} of BOTH batch elements (128 tokens each, 256 total).  Causal
attention work is then identical on every core (uniform SPMD program, with
per-core causality expressed in 0/1 mask *data*), and the only collective is
one 8-rank AllGather of locally-computed K|V per layer (Shared output).  The
tied LM head is vocab-sharded 8 ways after one final AllGather of activations.

All activations live in transposed [feature, token] layout so no on-device
transposes are ever needed; every DMA is an identity [128, F] copy (the host
pre-arranges all tensors into their exact SBUF image).  Matmuls run in bf16
with fp32 PSUM accumulation and an fp32 residual stream.

Perf structure (vs the original baseline):
- K and V are computed FIRST, one combined K|V AllGather (Shared output) is
  triggered, and Q + weight prefetches + dummy keep-warm matmuls (HAM clock)
  cover the collective; a tiny startup AllGather pre-warms the cc firmware.
- Attention batches all 8 key-rank chunks of one (batch, head) into a single
  [128, 1024] PSUM scores tile -> one Exp, one mask-multiply; softmax
  normalization uses reciprocal_approx_fast and runs one head behind the
  exp/AV stream; the v-bias is pre-added to V (equivalent post-softmax).
- LN1/LN2 weights/biases are folded into qkv_w/fc1_w host-side.
- Weight images are pair-fused (>=4KB per partition per DMA) and spread
  across the sync/scalar/gpsimd DMA queues; q/k are batch-major so the
  post-AllGather k_all loads are contiguous copies.
- The LM head makes emb stationary (N=512 streams over all 2048 tokens),
  loads each emb tile once, and writes bf16 logits.
"""

import sys

if "/opt/trn_rl_repo" not in sys.path:
    sys.path.insert(0, "/opt/trn_rl_repo")

import numpy as np
import ml_dtypes

BF16 = ml_dtypes.bfloat16

# Model config (hardcoded per problem spec)
V = 50257
D = 1024
H = 16
HD = 64
L = 8
FF = 4096
B = 2
T = 1024
SCALE = 1.0 / 8.0
EPS = 1e-5

N_CORES = 8
TL = 256         # local tokens per core (128 per batch element)
DT = D // 128    # 8 D-tiles
HP = H // 2      # 8 head-pairs
FT1 = FF // 128  # 32 fc1 output tiles
NV = 13          # vocab chunks of 512 per core
VPAD = NV * 512  # 6656 padded vocab slice per core
RG = [[0, 1, 2, 3, 4, 5, 6, 7]]

# packed per-layer bias/param tile columns (f32 [128, 72])
BC_QKVB = 0      # 24 cols (q 0:8 | k 8:16 | v 16:24)
BC_PROJB = 24    # 8
BC_FC1B = 32     # 32
BC_FC2B = 64     # 8
BCOLS = 72

KCOLS = HP * TL            # 2048 k cols in kv staging
VCOLS = 2 * H * 65         # 2080 v cols
KVCOLS = KCOLS + VCOLS     # 4128


def _build_program(n_layers=L, nv=NV, debug=False):
    import concourse.mybir as mybir
    import concourse.tile as tile
    from concourse import bacc
    from contextlib import ExitStack

    f32 = mybir.dt.float32
    f32r = mybir.dt.float32r
    bf16 = mybir.dt.bfloat16
    AF = mybir.ActivationFunctionType
    OP = mybir.AluOpType

    nc = bacc.Bacc("TRN2", target_bir_lowering=False, debug=False,
                   num_devices=N_CORES)

    # ---- external I/O (all pre-arranged host-side as SBUF images) ----
    # weight tiles are pair-fused so every DMA moves >=4KB per partition
    x0T = nc.dram_tensor("x0T", [128, DT * TL], f32, kind="ExternalInput")
    qkwT = nc.dram_tensor("qkwT", [n_layers, 8, 128, 2048], bf16, kind="ExternalInput")
    vwT = nc.dram_tensor("vwT", [n_layers, 4, 128, 2048], bf16, kind="ExternalInput")
    projwT = nc.dram_tensor("projwT", [n_layers, 4, 128, 2048], bf16, kind="ExternalInput")
    fc1wT = nc.dram_tensor("fc1wT", [n_layers, 16, 128, 2048], bf16, kind="ExternalInput")
    fc2wT = nc.dram_tensor("fc2wT", [n_layers, DT, 128, FT1 * 128], bf16, kind="ExternalInput")
    bias_in = nc.dram_tensor("biases", [n_layers, 128, BCOLS], f32, kind="ExternalInput")
    vbrow_in = nc.dram_tensor("vbrow", [n_layers, 1, 2 * 512], bf16, kind="ExternalInput")
    lnf_in = nc.dram_tensor("lnf", [128, 16], f32, kind="ExternalInput")
    masks_in = nc.dram_tensor("masks01", [128, 8 * 128], bf16, kind="ExternalInput")
    embT = nc.dram_tensor("embT", [nv, 4, 128, 1024], bf16, kind="ExternalInput")
    # out[v, vs, tp]: [128 vocab rows, 1024 token cols]
    out = nc.dram_tensor("out", [nv, 4, 2, 128, 1024], bf16, kind="ExternalOutput")
    dbg = {}
    if debug:
        for nm, shape, dt_ in [
            ("dbg_h1", [128, DT * TL], mybir.dt.bfloat16),
            ("dbg_q", [128, HP * TL], mybir.dt.bfloat16),
            ("dbg_k", [128, HP * TL], mybir.dt.bfloat16),
            ("dbg_vloc", [128, 2 * H * 65], mybir.dt.bfloat16),
            ("dbg_kall0", [128, 8 * 1024], mybir.dt.bfloat16),
            ("dbg_vtall0", [128, 8 * H * 65], mybir.dt.bfloat16),
            ("dbg_p00", [128, 8 * 128], mybir.dt.bfloat16),
            ("dbg_yT", [128, HP * TL], mybir.dt.bfloat16),
            ("dbg_x1", [128, DT * TL], f32),
            ("dbg_x2", [128, DT * TL], f32),
            ("dbg_hfT0", [128, 8 * 1024], mybir.dt.bfloat16),
            ("dbg_o00", [65, 128], f32),
            ("dbg_den00", [64, 128], f32),
        ]:
            dbg[nm] = nc.dram_tensor(nm, shape, dt_, kind="ExternalOutput")

    with tile.TileContext(nc) as tc:
        with ExitStack() as ctx:
            pool = lambda *a, **k: ctx.enter_context(tc.tile_pool(*a, **k))
            p_const = pool(name="const", bufs=1)
            p_x = pool(name="xres", bufs=1)
            p_h = pool(name="h", bufs=1)
            p_qk = pool(name="qk", bufs=1)
            p_vloc = pool(name="vloc", bufs=1)
            p_kvall = pool(name="kvall", bufs=2)
            p_y = pool(name="y", bufs=1)
            p_g = pool(name="g", bufs=1)
            p_wqk = pool(name="wqk", bufs=2)
            p_wv = pool(name="wv", bufs=4)
            p_wproj = pool(name="wproj", bufs=4)
            p_wfc1 = pool(name="wfc1", bufs=6)
            p_wfc2 = pool(name="wfc2", bufs=3)
            p_wemb = pool(name="wemb", bufs=4)
            p_bias = pool(name="bias", bufs=2)
            p_stat = pool(name="stat", bufs=1)
            p_ab = pool(name="ab", bufs=1)
            p_anr = pool(name="anr", bufs=4)
            p_scr = pool(name="scratch", bufs=3)
            ps_s = pool(name="ps_s", bufs=2, space="PSUM")
            ps_o = pool(name="ps_o", bufs=2, space="PSUM")
            ps_mm = pool(name="ps_mm", bufs=2, space="PSUM")
            p_dram = pool(name="dram", bufs=2, space="DRAM")

            # ---- preamble: constants ----
            masks = p_const.tile([128, 8 * 128], bf16)
            nc.sync.dma_start(masks[:], masks_in.ap())
            ones_f = p_const.tile([128, 1], bf16)
            nc.vector.memset(ones_f[:], 1.0)
            lnf = p_const.tile([128, 16], f32)
            nc.sync.dma_start(lnf[:], lnf_in.ap())

            x = p_x.tile([128, DT * TL], f32, tag="x")
            nc.sync.dma_start(x[:], x0T.ap())

            # Warm up the collectives firmware with a tiny dummy AllGather so
            # layer 0's real AllGather doesn't pay the ~75us first-call cost.
            cc_warm_in = p_dram.tile([128, 8], bf16, tag="cc_wi")
            nc.sync.dma_start(cc_warm_in[:], masks_in.ap()[:, 0:8])
            cc_warm_out = p_dram.tile([8 * 128, 8], bf16, tag="cc_wo",
                                      addr_space="Shared")
            nc.gpsimd.collective_compute(
                "AllGather", OP.bypass, ins=[cc_warm_in.opt()],
                outs=[cc_warm_out.opt()], replica_groups=RG)

            def pe_keep_warm(n, key):
                """Dummy matmuls so HAM doesn't re-throttle the PE during a
                collective wait.  ~210-430ns each; nothing reads the result."""
                warm_ps = ps_mm.tile([1, 512], f32, tag="mm",
                                     name=f"warm_{key}")
                for i in range(n):
                    nc.tensor.matmul(
                        warm_ps[:], ones_f[:], masks[:, 0:512],
                        start=True, stop=True, skip_group_check=True)

            def ln_stats(x_t):
                """Compute per-token (rstd, -mu*rstd) broadcast tiles."""
                stats1 = ps_o.tile([1, TL], f32, tag="o")
                stats2 = ps_o.tile([1, TL], f32, tag="o")
                for d in range(DT):
                    sl = slice(d * TL, (d + 1) * TL)
                    sc = p_scr.tile([128, 2 * TL], bf16, tag="sc16")
                    xb, xq = sc[:, 0:TL], sc[:, TL:2 * TL]
                    nc.scalar.copy(xb, x_t[:, sl])
                    nc.vector.tensor_mul(xq, x_t[:, sl], x_t[:, sl])
                    nc.tensor.matmul(stats1[:], ones_f[:], xb,
                                     start=(d == 0), stop=(d == DT - 1))
                    nc.tensor.matmul(stats2[:], ones_f[:], xq,
                                     start=(d == 0), stop=(d == DT - 1))
                # st cols: mu 0:TL | ex2 TL:2TL | var/sd 2TL:3TL | rstd 3TL:4TL
                st = p_stat.tile([1, 4 * TL], f32, tag="st")
                mu, ex2 = st[:, 0:TL], st[:, TL:2 * TL]
                vsd, rstd = st[:, 2 * TL:3 * TL], st[:, 3 * TL:4 * TL]
                nc.vector.tensor_scalar_mul(mu, stats1[:], 1.0 / D)
                nc.vector.tensor_scalar_mul(ex2, stats2[:], 1.0 / D)
                nc.vector.scalar_tensor_tensor(
                    vsd, mu, -1.0, mu, op0=OP.mult, op1=OP.mult)
                nc.vector.scalar_tensor_tensor(
                    vsd, vsd, EPS, ex2, op0=OP.add, op1=OP.add)
                nc.scalar.activation(vsd, vsd, AF.Sqrt, bias=0.0, scale=1.0)
                nc.vector.reciprocal(rstd, vsd)
                # cc = -mu * rstd overwrites mu
                nc.vector.scalar_tensor_tensor(
                    mu, mu, -1.0, rstd, op0=OP.mult, op1=OP.mult)
                ab = p_ab.tile([128, 2 * TL], f32, tag="ab")
                nc.gpsimd.partition_broadcast(ab[:, 0:TL], rstd)
                nc.gpsimd.partition_broadcast(ab[:, TL:2 * TL], mu)
                return ab

            def ln_apply(x_t, ab, h_t, w_ap=None, b_ap=None):
                """h = (x - mu) * rstd [* w + b], written bf16."""
                for d in range(DT):
                    sl = slice(d * TL, (d + 1) * TL)
                    tmp = p_scr.tile([128, TL], f32, tag="sc32")
                    nc.vector.tensor_mul(tmp[:], x_t[:, sl], ab[:, 0:TL])
                    if w_ap is None:
                        nc.vector.tensor_add(h_t[:, sl], tmp[:], ab[:, TL:2 * TL])
                    else:
                        nc.vector.tensor_add(tmp[:], tmp[:], ab[:, TL:2 * TL])
                        nc.scalar.activation(
                            h_t[:, sl], tmp[:], AF.Identity,
                            bias=b_ap[:, d:d + 1], scale=w_ap[:, d:d + 1])
                return h_t

            for l in range(n_layers):
                bias = p_bias.tile([128, BCOLS], f32, tag="bias")
                nc.sync.dma_start(bias[:], bias_in.ap()[l])
                # v-bias broadcast tile: adding b to V before AV is identical
                # to adding it to y after normalize (softmax weights sum to 1)
                vbr = p_bias.tile([1, 1024], bf16, tag="vbr", name=f"vbr_{l}", bufs=1)
                nc.sync.dma_start(vbr[:], vbrow_in.ap()[l])
                vb_b = p_bias.tile([128, 1024], bf16, tag="vb_b", name=f"vbb_{l}", bufs=1)
                nc.gpsimd.partition_broadcast(vb_b[:], vbr[:])

                # ---- LN1 (w/b folded into qkv weights host-side) ----
                ab1 = ln_stats(x)
                h1 = p_h.tile([128, DT * TL], bf16, tag="h", name=f"h1_{l}")
                ln_apply(x, ab1, h1)
                if debug and l == 0:
                    nc.sync.dma_start(dbg["dbg_h1"].ap(), h1[:])

                # ---- K first (so the collective can start early) ----
                # q_sb/k_sb are batch-major: col = b*1024 + hp*128 + i, so the
                # post-AllGather k_all loads are plain contiguous copies.
                q_sb = p_qk.tile([128, HP * TL], bf16, tag="q", name=f"q_{l}")
                k_sb = p_qk.tile([128, HP * TL], bf16, tag="k", name=f"k_{l}")

                def qk_tile(j, half, dst, wt, bcol):
                    """head-pair hp=4*?? f-tile f=2j+half from fused weight."""
                    hp = (2 * j + half) % 8
                    wbase = half * 1024
                    ps = ps_mm.tile([128, TL], f32, tag="mm",
                                    name=f"psqk_{l}_{j}_{half}_{bcol}")
                    for d in range(DT):
                        nc.tensor.matmul(
                            ps[:], wt[:, wbase + d * 128: wbase + (d + 1) * 128],
                            h1[:, d * TL:(d + 1) * TL],
                            start=(d == 0), stop=(d == DT - 1))
                    dstv = dst[:].rearrange("p (bb q) -> p bb q", bb=2)[
                        :, :, hp * 128:(hp + 1) * 128]
                    psv = ps[:].rearrange("p (bb q) -> p bb q", bb=2)
                    nc.scalar.activation(
                        dstv, psv, AF.Identity,
                        bias=bias[:, bcol:bcol + 1], scale=1.0)

                for j in range(4, 8):        # K head-pair fused tiles
                    wt = p_wqk.tile([128, 2048], bf16, tag="wqk",
                                    name=f"wqk_{l}_{j}")
                    nc.sync.dma_start(wt[:], qkwT.ap()[l, j])
                    for half in range(2):
                        qk_tile(j, half, k_sb, wt, BC_QKVB + 2 * j + half)

                # ---- V in [token, head_dim] layout with interleaved ones ----
                # v_loc cols: per b (1040): per head h (65): 64 v | 1 one
                wv = []
                for j in range(4):
                    w = p_wv.tile([128, 2048], bf16, tag="wv", name=f"wv_{l}_{j}")
                    nc.sync.dma_start(w[:], vwT.ap()[l, j])
                    wv.append(w)
                v_loc = p_vloc.tile([128, 2 * H * 65], bf16, tag="vloc",
                                    name=f"vloc_{l}")
                nc.vector.memset(
                    v_loc[:].rearrange("p (k c) -> p k c", c=65)[:, :, 64:65], 1.0)
                for b in range(2):
                    for hf in range(2):
                        ps_v = ps_mm.tile([128, 512], f32, tag="mm",
                                          name=f"ps_v_{l}_{b}_{hf}")
                        for d in range(DT):
                            nc.tensor.matmul(
                                ps_v[:],
                                h1[:, d * TL + b * 128: d * TL + b * 128 + 128],
                                wv[d // 2][:, (d % 2) * 1024 + hf * 512:
                                           (d % 2) * 1024 + (hf + 1) * 512],
                                start=(d == 0), stop=(d == DT - 1))
                        dst = v_loc[:, b * H * 65 + hf * 8 * 65:
                                    b * H * 65 + (hf * 8 + 8) * 65]
                        nc.vector.tensor_add(
                            dst.rearrange("p (h c) -> p h c", c=65)[:, :, 0:64],
                            ps_v[:].rearrange("p (h c) -> p h c", c=64),
                            vb_b[:, hf * 512:(hf + 1) * 512]
                            .rearrange("p (h c) -> p h c", c=64))

                # ---- stage K|V and AllGather (overlaps Q + weight loads) ----
                kv_in = p_dram.tile([128, KVCOLS], bf16, tag="kv_in",
                                    name=f"kv_in_{l}")
                nc.gpsimd.dma_start(kv_in[:, 0:KCOLS], k_sb[:])
                nc.sync.dma_start(kv_in[:, KCOLS:KVCOLS], v_loc[:])
                kv_out = p_dram.tile([8 * 128, KVCOLS], bf16, tag="kv_out",
                                     name=f"kv_out_{l}", addr_space="Shared")
                nc.gpsimd.collective_compute(
                    "AllGather", OP.bypass, ins=[kv_in.opt()],
                    outs=[kv_out.opt()], replica_groups=RG)

                # ---- Q (after the AG trigger, overlapping the collective) ----
                for j in range(4):
                    wt = p_wqk.tile([128, 2048], bf16, tag="wqk",
                                    name=f"wqk_{l}_{j}")
                    nc.sync.dma_start(wt[:], qkwT.ap()[l, j])
                    for half in range(2):
                        qk_tile(j, half, q_sb, wt, BC_QKVB + 2 * j + half)
                if debug and l == 0:
                    nc.sync.dma_start(dbg["dbg_q"].ap(), q_sb[:])
                    nc.sync.dma_start(dbg["dbg_k"].ap(), k_sb[:])
                    nc.sync.dma_start(dbg["dbg_vloc"].ap(), v_loc[:])

                # prefetch proj weights (keep DMA queues busy during AG wait)
                wp = []
                for j in range(4):
                    w = p_wproj.tile([128, 2048], bf16, tag="wproj",
                                     name=f"wproj_{l}_{j}")
                    nc.sync.dma_start(w[:], projwT.ap()[l, j])
                    wp.append(w)
                # prefetch fc1/fc2 weights during the AllGather + attention
                # window (DMA queues are otherwise idle there)
                wf1 = {}
                for j in range(6):
                    w = p_wfc1.tile([128, 2048], bf16, tag="wfc1",
                                    name=f"wfc1_{l}_{j}")
                    eng = nc.gpsimd if j % 2 == 0 else nc.sync
                    eng.dma_start(w[:], fc1wT.ap()[l, j])
                    wf1[j] = w
                wf2 = {}
                for f in range(1):
                    w = p_wfc2.tile([128, FT1 * 128], bf16, tag="wfc2",
                                    name=f"wfc2_{l}_{f}")
                    nc.sync.dma_start(w[:], fc2wT.ap()[l, f])
                    wf2[f] = w

                # keep the PE clock warm while waiting out the AllGather
                pe_keep_warm(140, f"ag_{l}")

                # ---- attention, one batch element at a time ----
                yT = p_y.tile([128, HP * TL], bf16, tag="yT", name=f"yT_{l}")
                for b in range(2):
                    # k_all cols: per rank rr (1024): per hp (128) -- the
                    # b-major k_sb layout makes this a contiguous copy
                    k_all = p_kvall.tile([128, 8 * 1024], bf16, tag="k_all",
                                         name=f"k_all_{l}_{b}", bufs=1)
                    for rr in range(8):
                        eng = (nc.gpsimd, nc.sync)[rr % 2]
                        eng.dma_start(
                            k_all[:, rr * 1024:(rr + 1) * 1024],
                            kv_out[rr * 128:(rr + 1) * 128,
                                   b * 1024:(b + 1) * 1024])
                    # vT_all cols: per rank rr (1040): per head h (65)
                    vT_all = p_kvall.tile([128, 8 * H * 65], bf16, tag="vT_all",
                                          name=f"vT_all_{l}_{b}", bufs=1)
                    for rr in range(8):
                        eng = (nc.sync, nc.gpsimd)[rr % 2]
                        eng.dma_start(
                            vT_all[:, rr * H * 65:(rr + 1) * H * 65],
                            kv_out[rr * 128:(rr + 1) * 128,
                                   KCOLS + b * H * 65:KCOLS + (b + 1) * H * 65])
                    if debug and l == 0 and b == 0:
                        nc.sync.dma_start(dbg["dbg_kall0"].ap(), k_all[:])
                        nc.sync.dma_start(dbg["dbg_vtall0"].ap(), vT_all[:])
                    # Epilogues run one head BEHIND the exp/AV stream so the
                    # ACT queue never stalls the next head's exp on this
                    # head's AV completing (den copy reads the AV result).
                    epi_q = []

                    def flush_epi():
                        h, o_ps, po, hp = epi_q.pop(0)
                        den = p_anr.tile([1, 2 * 128], f32, tag="den",
                                         name=f"den_{l}_{b}_{h}")
                        den_s, den_r = den[:, 0:128], den[:, 128:256]
                        nc.scalar.copy(den_s, o_ps[64:65, :])
                        nc.vector.reciprocal_approx_fast(den_r, den_s)
                        anr_b = p_anr.tile([64, 128], f32, tag="anr_b",
                                           name=f"anrb_{l}_{b}_{h}")
                        nc.gpsimd.partition_broadcast(anr_b[:], den_r)
                        ysl = yT[po:po + 64,
                                 hp * TL + b * 128: hp * TL + b * 128 + 128]
                        nc.vector.tensor_mul(ysl, o_ps[0:64, :], anr_b[:])

                    for hp in range(HP):
                        s_ps = []
                        for half in range(2):
                            po = 64 * half
                            sp = ps_s.tile([128, 8 * 128], f32, tag="s",
                                           name=f"s_{l}_{b}_{hp}_{half}")
                            for rr in range(8):
                                nc.tensor.matmul(
                                    sp[:, rr * 128:(rr + 1) * 128],
                                    k_all[po:po + 64, rr * 1024 + hp * 128:
                                          rr * 1024 + hp * 128 + 128],
                                    q_sb[po:po + 64, b * 1024 + hp * 128:
                                         b * 1024 + hp * 128 + 128],
                                    start=True, stop=True)
                            s_ps.append(sp)
                        for half in range(2):
                            h = 2 * hp + half
                            po = 64 * half
                            p_sb = p_scr.tile([128, 8 * 128], bf16, tag="sc16",
                                              name=f"p_{l}_{b}_{h}")
                            nc.scalar.activation(p_sb[:], s_ps[half][:], AF.Exp,
                                                 bias=0.0, scale=SCALE)
                            nc.vector.tensor_mul(p_sb[:], p_sb[:], masks[:])
                            if debug and l == 0 and b == 0 and h == 0:
                                nc.sync.dma_start(dbg["dbg_p00"].ap(), p_sb[:])
                            o_pool = ps_o if half == 0 else ps_mm
                            o_ps = o_pool.tile([65, 128], f32,
                                               tag=("o" if half == 0 else "mm"),
                                               name=f"o_{l}_{b}_{h}")
                            for rr in range(8):
                                nc.tensor.matmul(
                                    o_ps[:],
                                    vT_all[:, (rr * H + h) * 65:(rr * H + h) * 65 + 65],
                                    p_sb[:, rr * 128:(rr + 1) * 128],
                                    start=(rr == 0), stop=(rr == 7))
                            epi_q.append((h, o_ps, po, hp))
                            if len(epi_q) > 1:
                                flush_epi()
                    while epi_q:
                        flush_epi()

                if debug and l == 0:
                    nc.sync.dma_start(dbg["dbg_yT"].ap(), yT[:])
                # ---- proj + residual (in place on x) ----
                for f in range(DT):
                    ps = ps_mm.tile([128, TL], f32, tag="mm",
                                    name=f"pspj_{l}_{f}")
                    for k in range(DT):
                        nc.tensor.matmul(
                            ps[:], wp[f // 2][:, (f % 2) * 1024 + k * 128:
                                              (f % 2) * 1024 + (k + 1) * 128],
                            yT[:, k * TL:(k + 1) * TL],
                            start=(k == 0), stop=(k == DT - 1))
                    sl = slice(f * TL, (f + 1) * TL)
                    nc.vector.scalar_tensor_tensor(
                        x[:, sl], ps[:], bias[:, BC_PROJB + f:BC_PROJB + f + 1],
                        x[:, sl], op0=OP.add, op1=OP.add)

                if debug and l == 0:
                    nc.sync.dma_start(dbg["dbg_x1"].ap(), x[:])
                # ---- MLP (ln2 folded into fc1 weights host-side) ----
                ab2 = ln_stats(x)
                h2 = p_h.tile([128, DT * TL], bf16, tag="h", name=f"h2_{l}")
                ln_apply(x, ab2, h2)
                gT = p_g.tile([128, FT1 * TL], bf16, tag="gT", name=f"gT_{l}")
                for f in range(FT1):
                    j = f // 2
                    if f % 2 == 0:
                        if j in wf1:
                            wt = wf1.pop(j)
                        else:
                            wt = p_wfc1.tile([128, 2048], bf16, tag="wfc1",
                                             name=f"wfc1_{l}_{j}")
                            eng = nc.gpsimd if j % 2 == 0 else nc.sync
                            eng.dma_start(wt[:], fc1wT.ap()[l, j])
                        wf1_cur = wt
                    wt = wf1_cur
                    wbase = (f % 2) * 1024
                    ps = ps_mm.tile([128, TL], f32, tag="mm",
                                    name=f"psf1_{l}_{f}")
                    for d in range(DT):
                        nc.tensor.matmul(
                            ps[:], wt[:, wbase + d * 128: wbase + (d + 1) * 128],
                            h2[:, d * TL:(d + 1) * TL],
                            start=(d == 0), stop=(d == DT - 1))
                    nc.scalar.activation(
                        gT[:, f * TL:(f + 1) * TL], ps[:], AF.Gelu,
                        bias=bias[:, BC_FC1B + f:BC_FC1B + f + 1], scale=1.0)
                for f in range(DT):
                    ps = ps_mm.tile([128, TL], f32, tag="mm",
                                    name=f"psf2_{l}_{f}")
                    if f in wf2:
                        wt = wf2.pop(f)
                    else:
                        wt = p_wfc2.tile([128, FT1 * 128], bf16, tag="wfc2",
                                         name=f"wfc2_{l}_{f}")
                        eng = nc.gpsimd if f % 2 == 0 else nc.sync
                        eng.dma_start(wt[:], fc2wT.ap()[l, f])
                    for k in range(FT1):
                        nc.tensor.matmul(
                            ps[:], wt[:, k * 128:(k + 1) * 128],
                            gT[:, k * TL:(k + 1) * TL],
                            start=(k == 0), stop=(k == FT1 - 1))
                    sl = slice(f * TL, (f + 1) * TL)
                    nc.vector.scalar_tensor_tensor(
                        x[:, sl], ps[:], bias[:, BC_FC2B + f:BC_FC2B + f + 1],
                        x[:, sl], op0=OP.add, op1=OP.add)

            if debug:
                nc.sync.dma_start(dbg["dbg_x2"].ap(), x[:])
            # ---- final LN + AllGather + vocab-sharded tied head ----
            abf = ln_stats(x)
            hf = p_h.tile([128, DT * TL], bf16, tag="h", name="hf")
            ln_apply(x, abf, hf, w_ap=lnf[:, 0:8], b_ap=lnf[:, 8:16])
            hf_in = p_dram.tile([128, DT * TL], bf16, tag="kv_in", name="hf_in")
            nc.sync.dma_start(hf_in[:], hf[:])
            hf_out = p_dram.tile([8 * 128, DT * TL], bf16, tag="kv_out",
                                 name="hf_out", addr_space="Shared")
            nc.gpsimd.collective_compute(
                "AllGather", OP.bypass, ins=[hf_in.opt()], outs=[hf_out.opt()],
                replica_groups=RG)

            pe_keep_warm(160, "hf")
            # hfT: [dim-of-dtile, 2048 global tokens], token col = rr*256+b*128+i
            hfT = [p_kvall.tile([128, 8 * 1024], bf16,
                                tag=("k_all" if g == 0 else "vT_all"),
                                name=f"hfT_{g}", bufs=1) for g in range(2)]
            for d in range(DT):
                for rr in range(8):
                    nc.gpsimd.dma_start(
                        hfT[d // 4][:, (d % 4) * 2048 + rr * 256:
                                    (d % 4) * 2048 + rr * 256 + 256],
                        hf_out[rr * 128:(rr + 1) * 128, d * TL:(d + 1) * TL])
            if debug:
                nc.sync.dma_start(dbg["dbg_hfT0"].ap(), hfT[0][:])

            for v in range(nv):
                em = []
                for j in range(4):
                    e = p_wemb.tile([128, 1024], bf16, tag="emb",
                                    name=f"emb_{v}_{j}")
                    nc.sync.dma_start(e[:], embT.ap()[v, j])
                    em.append(e)
                for vs in range(4):
                    for tp in range(2):
                        acc = ps_s.tile([128, 1024], f32, tag="s",
                                        name=f"acc_{v}_{vs}_{tp}")
                        for d in range(DT):
                            for c2 in range(2):
                                nc.tensor.matmul(
                                    acc[:, c2 * 512:(c2 + 1) * 512],
                                    em[d // 2][:, (d % 2) * 512 + vs * 128:
                                               (d % 2) * 512 + (vs + 1) * 128],
                                    hfT[d // 4][:, (d % 4) * 2048 + tp * 1024
                                                + c2 * 512:
                                                (d % 4) * 2048 + tp * 1024
                                                + (c2 + 1) * 512],
                                    start=(d == 0), stop=(d == DT - 1))
                        osb = p_scr.tile([128, 1024], bf16, tag="sc16",
                                         name=f"osb_{v}_{vs}_{tp}")
                        nc.scalar.copy(osb[:], acc[:])
                        nc.gpsimd.dma_start(out.ap()[v, vs, tp], osb[:])

    nc.compile()
    return nc


def _sbuf_image(wT, ft):
    """[K, F] (already transposed weight) -> [F//ft, 128, (K//128)*ft]."""
    K, F = wT.shape
    return np.ascontiguousarray(
        wT.reshape(K // 128, 128, F // ft, ft).transpose(2, 1, 0, 3)
        .reshape(F // ft, 128, (K // 128) * ft))


def prep_inputs(inputs, n_layers=L, nv=NV):
    """Build the 8 per-core input maps from full model inputs."""
    idx = np.asarray(inputs["idx"]).astype(np.int64)
    tok_emb = np.asarray(inputs["tok_emb"], dtype=np.float32)
    pos_emb = np.asarray(inputs["pos_emb"], dtype=np.float32)
    x0 = tok_emb[idx] + pos_emb[0, :T][None, :, :]    # [B, T, D] f32

    vpad = nv * 512
    lnf_w = np.asarray(inputs["lnf_w"], dtype=np.float32)
    emb_pad = np.zeros((N_CORES * vpad, D), dtype=np.float32)
    emb_pad[:min(N_CORES * vpad, V)] = tok_emb[:min(N_CORES * vpad, V)]
    emb_pad *= lnf_w[None, :]      # fold lnf_w into the tied head

    def _fuse(img):
        """[F, 128, C] -> [F//2, 128, 2C] pairing consecutive f-tiles."""
        F, P, C = img.shape
        return np.ascontiguousarray(
            img.reshape(F // 2, 2, P, C).transpose(0, 2, 1, 3)
            .reshape(F // 2, P, 2 * C))

    shared = {}
    qkw = np.empty((n_layers, 8, 128, 2048), dtype=BF16)
    vw = np.empty((n_layers, 4, 128, 2048), dtype=BF16)
    qkvb = np.empty((n_layers, 3 * D), dtype=np.float32)
    fc1b = np.empty((n_layers, FF), dtype=np.float32)
    fc1w_img = np.empty((n_layers, 16, 128, 2048), dtype=BF16)
    for l in range(n_layers):
        w1 = np.asarray(inputs["ln1_w"][l], dtype=np.float32)
        b1 = np.asarray(inputs["ln1_b"][l], dtype=np.float32)
        w2 = np.asarray(inputs["ln2_w"][l], dtype=np.float32)
        b2 = np.asarray(inputs["ln2_b"][l], dtype=np.float32)
        qw = np.asarray(inputs["qkv_w"][l], dtype=np.float32)   # [3D, D]
        qkvb[l] = np.asarray(inputs["qkv_b"][l], dtype=np.float32) + qw @ b1
        qwf = qw * w1[None, :]                                  # fold ln1_w
        wT = qwf.T                                              # [D, 3D]
        qkw[l] = _fuse(_sbuf_image(wT[:, :2 * D].astype(BF16), 128))
        vw[l] = _fuse(wT[:, 2 * D:].astype(BF16).reshape(DT, 128, 1024))
        f1 = np.asarray(inputs["fc1_w"][l], dtype=np.float32)   # [FF, D]
        fc1b[l] = np.asarray(inputs["fc1_b"][l], dtype=np.float32) + f1 @ b2
        fc1w_img[l] = _fuse(_sbuf_image((f1 * w2[None, :]).T.astype(BF16), 128))
    shared["qkwT"] = qkw
    shared["vwT"] = vw
    shared["fc1wT"] = fc1w_img
    shared["projwT"] = np.stack([
        _fuse(_sbuf_image(np.asarray(inputs["proj_w"][l], dtype=np.float32).T.astype(BF16), 128))
        for l in range(n_layers)])
    shared["fc2wT"] = np.stack([
        _sbuf_image(np.asarray(inputs["fc2_w"][l], dtype=np.float32).T.astype(BF16), 128)
        for l in range(n_layers)])

    def cols(a, n):
        return a.reshape(n_layers, n, 128).transpose(0, 2, 1)

    shared["vbrow"] = np.ascontiguousarray(
        qkvb[:, 2 * D:].reshape(n_layers, 1, D).astype(BF16))
    biases = np.zeros((n_layers, 128, BCOLS), dtype=np.float32)
    biases[:, :, BC_QKVB:BC_QKVB + 24] = cols(qkvb, 24)
    biases[:, :, BC_PROJB:BC_PROJB + 8] = cols(
        np.asarray(inputs["proj_b"], dtype=np.float32)[:n_layers], 8)
    biases[:, :, BC_FC1B:BC_FC1B + 32] = cols(fc1b, 32)
    biases[:, :, BC_FC2B:BC_FC2B + 8] = cols(
        np.asarray(inputs["fc2_b"], dtype=np.float32)[:n_layers], 8)
    shared["biases"] = np.ascontiguousarray(biases)
    lnf = np.zeros((128, 16), dtype=np.float32)
    lnf[:, 0:8] = np.ones((D,), dtype=np.float32).reshape(8, 128).T
    lnf[:, 8:16] = np.asarray(inputs["lnf_b"], dtype=np.float32).reshape(8, 128).T
    shared["lnf"] = lnf

    tri_incl = np.tril(np.ones((128, 128), dtype=np.float32)).T  # [kt,q] kt<=q
    tri_excl = np.tril(np.ones((128, 128), dtype=np.float32), -1).T  # kt<q

    ii = np.arange(128)
    in_maps = []
    for c in range(N_CORES):
        m = dict(shared)
        xl = np.concatenate([x0[0, 8 * ii + c], x0[1, 8 * ii + c]], 0)  # [256, D]
        m["x0T"] = np.ascontiguousarray(
            xl.T.reshape(DT, 128, TL).transpose(1, 0, 2).reshape(128, DT * TL))
        mk = np.concatenate(
            [tri_incl if rr <= c else tri_excl for rr in range(8)], 1)
        m["masks01"] = np.ascontiguousarray(mk.astype(BF16))
        esl = emb_pad[c * vpad:(c + 1) * vpad]        # [vpad, D]
        # embT[v, j] = [128, 1024]: d-pair fused images of [dims, 512 vocab]
        et = esl.T.astype(BF16).reshape(DT, 128, nv, 512).transpose(2, 0, 1, 3)
        m["embT"] = np.ascontiguousarray(
            et.reshape(nv, 4, 2, 128, 512).transpose(0, 1, 3, 2, 4)
            .reshape(nv, 4, 128, 1024))
        in_maps.append(m)
    return in_maps


def assemble_output(results, nv=NV):
    """results: list of 8 dicts with 'out' [nv, 4, 2, 128, 1024] bf16."""
    vpad = nv * 512
    logits = np.empty((B, T, V), dtype=np.float32)
    for c in range(N_CORES):
        o = np.asarray(results[c]["out"], dtype=np.float32)
        # -> [vpad vocab, 2048 token-cols]
        o = o.transpose(0, 1, 3, 2, 4).reshape(vpad, 2048)
        lo = c * vpad
        hi = min((c + 1) * vpad, V)
        if hi <= lo:
            continue
        # token col j: rr = j//256, b = (j%256)//128, i = j%128 -> tok 8i+rr
        ocols = o[:hi - lo].reshape(hi - lo, 8, 2, 128)  # [vv, rr, b, i]
        for b in range(B):
            for rr in range(8):
                g = 8 * np.arange(128) + rr
                logits[b, g, lo:hi] = ocols[:, rr, b, :].T
    return logits


_prog_cache = {}


def _get_program(n_layers=L, nv=NV, debug=False):
    key = (n_layers, nv, debug)
    if key not in _prog_cache:
        _prog_cache[key] = _build_program(n_layers, nv, debug)
    return _prog_cache[key]


def run(inputs, n_layers=L, nv=NV, trace=False, debug=False):
    from concourse import bass_utils
    nc = _get_program(n_layers, nv, debug)
    in_maps = prep_inputs(inputs, n_layers, nv)
    res = bass_utils.run_bass_kernel_spmd(
        nc, in_maps, core_ids=list(range(N_CORES)), trace=trace)
    return assemble_output(res.results, nv), res


def kernel(**inputs):
    logits, _ = run(inputs)
    return logits
